# revision 2
# baseline (speedup 1.0000x reference)
"""LoRA layer kernel for Trainium2, SPMD across 8 NeuronCores.

Computes: out[b,s,h,d] = x[b,s,:] @ W_orig[:,h,d] + SCALE * (x @ A) @ B[:,h,d]

Strategy (data-parallel over tokens, per the sharding hint's DP branch):
  - Fold LoRA into the weights on the host: W_eff = W + (SCALE*A) @ B
    (exact by associativity; a 33 MFLOP host-side GEMM vs the 68.7 GFLOP
    main matmul which stays on device).
  - Cast x and W_eff to fp16 on the host: halves DMA traffic and runs the
    PE at 1 col/cycle (4x the fp32 rate) with fp32 PSUM accumulation.
    Output is written fp16 and upcast on the host (error ~1e-3 << 2e-2).
  - Shard x over tokens (8192 -> 1024 per core); W_eff replicated.
  - Per core: out[1024, 2048] = x_slice @ W_eff, K=2048 contraction in
    16 k-tiles of 128. Token tiles run in PAIRS sharing all 8 PSUM banks
    with the k-loop OUTER within a pair, so the first pair's compute
    (27 us) overlaps the W_eff stream-in (24 us) k-tile by k-tile.
  - x arrives host-pretransposed as [t, p, k, tt] so each DMA line is
    4KB contiguous and the contraction dim lands on SBUF partitions.
"""

import numpy as np

# Problem shapes (hardcoded per contract - kernel.py must be self-contained)
B, S, H = 4, 2048, 2048
NH, HD = 16, 128
N = NH * HD            # 2048 output features
RANK = 4
ALPHA = 4.0
SCALE = ALPHA / RANK   # 1.0
NCORES = 8
TOK = B * S            # 8192 tokens total
TPC = TOK // NCORES    # 1024 tokens per core

P = 128                # SBUF partitions
KT = H // P            # 16 contraction tiles
TT = TPC // P          # 8 token tiles per core
CH = 512               # psum chunk width (one fp32 PSUM bank)
NCH = N // CH          # 4 chunks

_CACHE = {}


def _build_program():
    import concourse.mybir as mybir
    import concourse.tile as tile
    from concourse import bacc

    f16 = mybir.dt.float16
    f32 = mybir.dt.float32

    nc = bacc.Bacc(None, target_bir_lowering=False, debug=False)

    xt = nc.dram_tensor("xt", [TT, P, KT, P], f16, kind="ExternalInput")
    w = nc.dram_tensor("w", [P, KT, N], f16, kind="ExternalInput")
    out = nc.dram_tensor("out", [TT, P, N], f16, kind="ExternalOutput")

    with tile.TileContext(nc) as tc:
        with (
            tc.tile_pool(name="wpool", bufs=1) as wpool,
            tc.tile_pool(name="xpool", bufs=4) as xpool,
            tc.tile_pool(name="opool", bufs=4) as opool,
            tc.tile_pool(name="psum", bufs=8, space="PSUM") as psum,
        ):
            # x token tiles: prefetched rotating through 4 buffers
            x_tiles = {}

            def load_x(t):
                xr = xpool.tile([P, KT, P], f16, tag="x", name=f"x_{t}")
                nc.sync.dma_start(xr[:], xt[t])
                x_tiles[t] = xr

            load_x(0)
            load_x(1)

            # W_eff k-tiles: resident for the whole kernel, streamed in
            # one DMA per k-tile so the first pair's k-loop can chase the
            # arrival front.
            w_tiles = []
            for k in range(KT):
                wk = wpool.tile([P, N], f16, tag=f"w{k}", name=f"w_{k}")
                nc.sync.dma_start(wk[:], w[:, k, :])
                w_tiles.append(wk)

            for pr in range(TT // 2):
                ta, tb = 2 * pr, 2 * pr + 1
                if ta + 2 < TT:
                    load_x(ta + 2)
                if tb + 2 < TT:
                    load_x(tb + 2)
                pss = {
                    (t, c): psum.tile([P, CH], f32, tag="ps",
                                      name=f"ps_{t}_{c}")
                    for t in (ta, tb) for c in range(NCH)
                }
                for k in range(KT):
                    for t in (ta, tb):
                        lhsT = x_tiles[t][:, k, :]
                        for c in range(NCH):
                            nc.tensor.matmul(
                                pss[(t, c)][:],
                                lhsT,
                                w_tiles[k][:, c * CH:(c + 1) * CH],
                                start=(k == 0), stop=(k == KT - 1),
                            )
                for t in (ta, tb):
                    ot = opool.tile([P, N], f16, tag="o", name=f"o_{t}")
                    for c in range(NCH):
                        # split evacuation across both free engines
                        eng = nc.vector if c % 2 == 0 else nc.scalar
                        if c % 2 == 0:
                            eng.tensor_copy(ot[:, c * CH:(c + 1) * CH],
                                            pss[(t, c)][:])
                        else:
                            eng.copy(ot[:, c * CH:(c + 1) * CH],
                                     pss[(t, c)][:])
                    nc.sync.dma_start(out[t], ot[:])
                for t in (ta, tb):
                    x_tiles.pop(t)

    nc.compile()
    return nc


def _prep_inputs(x, W_orig, A_kernel, B_kernel):
    x = np.asarray(x, dtype=np.float32)
    W_orig = np.asarray(W_orig, dtype=np.float32)
    A_kernel = np.asarray(A_kernel, dtype=np.float32)
    B_kernel = np.asarray(B_kernel, dtype=np.float32)

    # Fold the rank-4 LoRA path into the dense weight (exact algebra):
    # out = x @ (W + SCALE * A @ B)
    w_eff = W_orig.reshape(H, N) + (SCALE * A_kernel) @ B_kernel.reshape(RANK, N)
    # device layout [p, k, n] with h = k*128 + p
    w_dev = np.ascontiguousarray(
        w_eff.reshape(KT, P, N).transpose(1, 0, 2)).astype(np.float16)

    x2 = x.reshape(TOK, H)
    in_maps = []
    for i in range(NCORES):
        xs = x2[i * TPC:(i + 1) * TPC]                      # [1024, 2048]
        # [t, tt, k, p] -> [t, p, k, tt] so the contraction dim is the
        # SBUF partition dim and each DMA partition-line is contiguous
        xd = np.ascontiguousarray(
            xs.reshape(TT, P, KT, P).transpose(0, 3, 2, 1)).astype(np.float16)
        in_maps.append({"xt": xd, "w": w_dev})
    return in_maps


def kernel(x, W_orig, A_kernel, B_kernel):
    from concourse.bass_utils import run_bass_kernel_spmd

    if "nc" not in _CACHE:
        _CACHE["nc"] = _build_program()
    nc = _CACHE["nc"]

    in_maps = _prep_inputs(x, W_orig, A_kernel, B_kernel)
    res = run_bass_kernel_spmd(nc, in_maps, list(range(NCORES)))
    parts = [
        res.results[i]["out"].reshape(TPC, N).astype(np.float32)
        for i in range(NCORES)
    ]
    full = np.concatenate(parts, axis=0)                    # [TOK, N]
    return full.reshape(B, S, NH, HD)
